# revision 2
# baseline (speedup 1.0000x reference)
"""Trainium2 Bass kernel for rank-1-projection attention.

Computation (all fp32):
    q = x_q @ WQ            [512,512,256]@[256] -> [512,512]
    k = x_k @ WK
    v = x_v @ WV
    y = softmax(q @ k, axis=-1) @ v     -> [512,512]

Strategy: data-parallel over the leading N axis (64 rows/core x 8 cores).
The projections are the bulk: 100.7 MB of HBM reads per core (DMA floor
~281 us at 358 GB/s/core).  Each [128, 256] row-block x W dot-product is
ONE fused DVE tensor_tensor_reduce (mult + free-axis reduce in a single
pass, ~(58+256)/0.96GHz = 327 ns), writing the projected column directly
into the transposed layouts the tensor-engine matmuls need.  Total DVE
time ~251 us < DMA, so the kernel is DMA-bound.
k/v rows are AllGathered ([64,1024] -> [512,1024], overlapped with the
q projection) and the tiny attention chain runs per-core on its 64 rows.
"""

import numpy as np

import concourse.bass as bass
import concourse.mybir as mybir
import concourse.tile as tile
from concourse import bacc
from concourse.bass_utils import run_bass_kernel_spmd
from concourse.masks import make_identity

N = 512          # attention size (rows/cols)
D = 256          # projection dim
CORES = 8
NL = N // CORES  # 64 leading rows per core
R = NL * N       # 32768 projection rows per tensor per core
G = 32           # leading-index count per DMA tile (4 MB tiles)
NBLK = N // 128  # 4: 128-blocks of the inner axis

F32 = mybir.dt.float32

_CACHE = {}


def _build():
    key = "nc"
    if key in _CACHE:
        return _CACHE[key]

    nc = bacc.Bacc(
        "TRN2", target_bir_lowering=False, debug=False, num_devices=CORES
    )

    xq = nc.dram_tensor("xq", [R, D], F32, kind="ExternalInput")
    xk = nc.dram_tensor("xk", [R, D], F32, kind="ExternalInput")
    xv = nc.dram_tensor("xv", [R, D], F32, kind="ExternalInput")
    wall = nc.dram_tensor("wall", [128, 3, D], F32, kind="ExternalInput")
    yout = nc.dram_tensor("yout", [NL, N], F32, kind="ExternalOutput")

    with tile.TileContext(nc) as tc:
        with (
            tc.tile_pool(name="consts", bufs=1) as consts,
            tc.tile_pool(name="xs", bufs=3) as xs_pool,
            tc.tile_pool(name="scr", bufs=2) as scr_pool,
            tc.tile_pool(name="small", bufs=1) as small,
            tc.tile_pool(name="psum", bufs=1, space="PSUM") as psum_pool,
            tc.tile_pool(name="dram", bufs=1, space="DRAM") as dram_pool,
        ):
            w_tile = consts.tile([128, 3, D], F32)
            nc.scalar.dma_start(w_tile[:], wall[:])
            ident = consts.tile([128, 128], F32)
            make_identity(nc, ident[:])

            # Transposed projection outputs: xt[b][p, c] = proj[c, 128*b + p]
            qt = [consts.tile([128, NL], F32, name=f"qt{b}") for b in range(NBLK)]
            kt = [consts.tile([128, NL], F32, name=f"kt{b}") for b in range(NBLK)]
            vt = [consts.tile([128, NL], F32, name=f"vt{b}") for b in range(NBLK)]

            def project(x_dram, widx, dest):
                # row r = 512*c + 128*b + p  (c = leading index, b = inner
                # 128-block, p = partition). One DMA tile = fixed b, G
                # c-values.  Per c: one fused DVE tensor_tensor_reduce
                # (x * W then reduce over d) -> dest[b][:, c].
                x4 = x_dram.rearrange("(c b p) d -> b p c d", p=128, b=NBLK)
                for b in range(NBLK):
                    for jc in range(NL // G):
                        xtile = xs_pool.tile([128, G, D], F32, tag="xtile", name="xtile")
                        nc.sync.dma_start(xtile[:], x4[b, :, jc * G : (jc + 1) * G])
                        for c in range(G):
                            scr = scr_pool.tile([128, 1, D], F32, tag="scr", name="scr")
                            nc.vector.tensor_tensor_reduce(
                                out=scr[:],
                                in0=xtile[:, c : c + 1, :],
                                in1=w_tile[:, widx : widx + 1, :],
                                scale=1.0,
                                scalar=0.0,
                                op0=mybir.AluOpType.mult,
                                op1=mybir.AluOpType.add,
                                accum_out=dest[b][:, jc * G + c : jc * G + c + 1],
                            )

            # ---- k and v projections first so the AllGather can overlap q ----
            project(xk, 1, kt)
            project(xv, 2, vt)

            # kv_loc[m_local, 0:512] = k rows, [m_local, 512:1024] = v rows
            kv_loc = small.tile([NL, 2 * N], F32)
            for b in range(NBLK):
                pk = psum_pool.tile([NL, 128], F32, tag="tp", bufs=2, name="pk")
                nc.tensor.transpose(pk[:], kt[b][:], ident[:])
                nc.vector.tensor_copy(out=kv_loc[:, b * 128 : (b + 1) * 128], in_=pk[:])
            for b in range(NBLK):
                pv = psum_pool.tile([NL, 128], F32, tag="tp", bufs=2, name="pv")
                nc.tensor.transpose(pv[:], vt[b][:], ident[:])
                nc.vector.tensor_copy(
                    out=kv_loc[:, N + b * 128 : N + (b + 1) * 128], in_=pv[:]
                )

            cc_in = dram_pool.tile([NL, 2 * N], F32)
            cc_out = dram_pool.tile([N, 2 * N], F32, addr_space="Shared")
            nc.sync.dma_start(cc_in[:], kv_loc[:])
            nc.gpsimd.collective_compute(
                "AllGather",
                mybir.AluOpType.bypass,
                replica_groups=[list(range(CORES))],
                ins=[cc_in[:].opt()],
                outs=[cc_out[:].opt()],
            )

            # ---- q projection (overlaps with the AllGather) ----
            project(xq, 0, qt)

            # kv_full[b][p, 0:512]=k[128b+p, :], [p, 512:1024]=v[128b+p, :]
            # issued on the ACT hwdge ring so waiting on the collective does
            # not head-of-line-block the sync ring streaming x_q tiles.
            kv_full = [
                consts.tile([128, 2 * N], F32, name=f"kv{b}") for b in range(NBLK)
            ]
            for b in range(NBLK):
                nc.scalar.dma_start(kv_full[b][:], cc_out[b * 128 : (b + 1) * 128, :])

            # ---- attention tail ----
            py = psum_pool.tile([NL, N], F32, tag="mm", name="py")
            for b in range(NBLK):
                nc.tensor.matmul(
                    py[:],
                    lhsT=qt[b][:],
                    rhs=kv_full[b][:, 0:N],
                    start=(b == 0),
                    stop=(b == NBLK - 1),
                )

            neg_mx = small.tile([NL, 1], F32)
            nc.vector.tensor_reduce(
                out=neg_mx[:], in_=py[:], axis=mybir.AxisListType.X,
                op=mybir.AluOpType.max, negate=True,
            )
            s_sb = small.tile([NL, N], F32)
            sumexp = small.tile([NL, 1], F32)
            nc.scalar.activation(
                s_sb[:], py[:], mybir.ActivationFunctionType.Exp,
                bias=neg_mx[:], scale=1.0, accum_out=sumexp[:],
            )
            rsum = small.tile([NL, 1], F32)
            nc.vector.reciprocal(rsum[:], sumexp[:])

            st = [consts.tile([128, NL], F32, name=f"st{b}") for b in range(NBLK)]
            for b in range(NBLK):
                ps = psum_pool.tile([128, NL], F32, tag="tp2", bufs=2, name="ps")
                nc.tensor.transpose(
                    ps[:], s_sb[:, b * 128 : (b + 1) * 128], ident[:NL, :NL]
                )
                nc.vector.tensor_copy(out=st[b][:], in_=ps[:])

            po = psum_pool.tile([NL, N], F32, tag="mm", name="po")
            for b in range(NBLK):
                nc.tensor.matmul(
                    po[:],
                    lhsT=st[b][:],
                    rhs=kv_full[b][:, N : 2 * N],
                    start=(b == 0),
                    stop=(b == NBLK - 1),
                )

            out_sb = small.tile([NL, N], F32)
            nc.vector.tensor_scalar_mul(out_sb[:], po[:], rsum[:])
            nc.sync.dma_start(yout[:], out_sb[:])

    nc.compile()
    _CACHE[key] = nc
    return nc


def _make_in_maps(inputs):
    x_q = np.asarray(inputs["x_q"], dtype=np.float32)
    x_k = np.asarray(inputs["x_k"], dtype=np.float32)
    x_v = np.asarray(inputs["x_v"], dtype=np.float32)
    w_all = np.stack(
        [
            np.tile(np.asarray(inputs["WQ"], dtype=np.float32), (128, 1)),
            np.tile(np.asarray(inputs["WK"], dtype=np.float32), (128, 1)),
            np.tile(np.asarray(inputs["WV"], dtype=np.float32), (128, 1)),
        ],
        axis=1,
    )  # [128, 3, D]
    in_maps = []
    for r in range(CORES):
        sl = slice(r * NL, (r + 1) * NL)
        in_maps.append(
            {
                "xq": np.ascontiguousarray(x_q[sl]).reshape(R, D),
                "xk": np.ascontiguousarray(x_k[sl]).reshape(R, D),
                "xv": np.ascontiguousarray(x_v[sl]).reshape(R, D),
                "wall": w_all,
            }
        )
    return in_maps


def _run(inputs, trace=False):
    nc = _build()
    res = run_bass_kernel_spmd(
        nc, _make_in_maps(inputs), core_ids=list(range(CORES)), trace=trace
    )
    out = np.concatenate(
        [res.results[r]["yout"] for r in range(CORES)], axis=0
    ).astype(np.float32)
    return out, res


def kernel(**inputs):
    out, _ = _run(inputs)
    return out
